# revision 53
# baseline (speedup 1.0000x reference)
"""Trainium2 Bass kernel for nn_Block_55336358643145 (dense transformer block).

v2: head-sharded exact-causal attention.

Row-shards the 4096 (batch*seq) rows contiguously: core c owns rows
512c..512(c+1) (cores 0-3 batch 0, 4-7 batch 1). Per core: LN1 ->
transpose -> Q/K/V projections emitted in output-chunk-major order so
chunk oc (128 dims = heads 2oc,2oc+1) is the AllToAll payload for core
oc. Two AllToAlls (k+v early, q after) re-shard to head-parallel: core j
holds q/k/v for heads 2j,2j+1 over all 4096 rows. Attention is then an
exact causal triangle, identical on every core (shared triangular mask,
shared per-ktile column-zero exp bias, shrinking q-windows on diagonal
tiles), with a per-head ones column in v producing the softmax
denominator. Normalized outputs AllToAll back to row-sharding, then
out-proj + residual, LN2, 4x MLP with exact-erf Gelu as before. All
matmuls bf16.
"""

import contextlib

import numpy as np

import concourse.bass as bass
import concourse.tile as tile
from concourse import bacc, mybir
from concourse.bass_utils import run_bass_kernel_spmd

F32 = mybir.dt.float32
BF16 = mybir.dt.bfloat16
FP8 = mybir.dt.float8e4
AF = mybir.ActivationFunctionType
ALU = mybir.AluOpType

B, S, D, H, HD, FF = 2, 2048, 1024, 16, 64, 4096
NCORE = 8
R = 512            # rows per core
DC = D // 128      # 8 d-chunks
GC = FF // 128     # 32 mlp hidden chunks
VW = H * (HD + 1)  # 1040: v with per-head ones column
CW = 4 * (HD + 1) * 2  # 520: v cols per partition in an A2A chunk
LN_EPS = 1e-5
JD = 25            # joined dim for the column-zero mask
NEG = -1.0e30
GROUPS = [[0, 1, 2, 3, 4, 5, 6, 7]]


def build_program(apply_bv, apply_ln1_gb, apply_ln2_gb):
    nc = bacc.Bacc("TRN2", target_bir_lowering=False, debug=False,
                   num_devices=NCORE)

    def inp(name, shape):
        return nc.dram_tensor(name, list(shape), F32, kind="ExternalInput").ap()

    def binp(name, shape):
        return nc.dram_tensor(name, list(shape), BF16,
                              kind="ExternalInput").ap()

    io = dict(
        hs=inp("hs", (R, D)),
        wq=binp("wq", (D, D)), wk=binp("wk", (D, D)),
        wv=binp("wv", (D, D)), wp=binp("wp", (D, D)),
        w1=binp("w1", (GC, 128, DC, 128)), w2=binp("w2", (FF, D)),
        bq8=inp("bq8", (128, DC)), bkl=inp("bkl", (128, DC)),
        bvh2=inp("bvh2", (HD, 2)), b1l=inp("b1l", (128, GC)),
        bpr=binp("bpr", (1, D)), b2r=binp("b2r", (1, D)),
        ln1gb=inp("ln1gb", (2, D)), ln2gb=inp("ln2gb", (2, D)),
        colz=inp("colz", (128, 16)),
        tri=inp("tri", (128, 128)),
        ident=inp("ident", (128, 128)),
        onesr=binp("onesr", (1, 128)),
        vones=nc.dram_tensor("vones", [128, H, 1], FP8,
                             kind="ExternalInput").ap(),
        out=nc.dram_tensor("out", [R, D], F32, kind="ExternalOutput").ap(),
    )

    with tile.TileContext(nc) as tc:
        _build(tc, io, apply_bv, apply_ln1_gb, apply_ln2_gb)
    nc.compile()
    return nc


def _build(tc, io, apply_bv, apply_ln1_gb, apply_ln2_gb):
    nc = tc.nc
    hs, out = io["hs"], io["out"]

    with contextlib.ExitStack() as ctx:
        persist = ctx.enter_context(tc.tile_pool(name="persist", bufs=1,
                                                 side="left"))
        dram = ctx.enter_context(tc.tile_pool(name="dram", bufs=1,
                                              space="DRAM"))

        # ---- small constants ------------------------------------------------
        ident_sb = persist.tile([128, 128], F32)
        nc.sync.dma_start(ident_sb[:], io["ident"][:])
        eps_sb = persist.tile([128, 1], F32)
        nc.vector.memset(eps_sb[:], LN_EPS)
        ones_r = persist.tile([1, 128], BF16)
        nc.sync.dma_start(ones_r[:], io["onesr"][:])
        bq8_sb = persist.tile([128, DC], F32)
        nc.sync.dma_start(bq8_sb[:], io["bq8"][:])
        bkl_sb = persist.tile([128, DC], F32)
        nc.sync.dma_start(bkl_sb[:], io["bkl"][:])
        b1l_sb = persist.tile([128, GC], F32)
        nc.sync.dma_start(b1l_sb[:], io["b1l"][:])
        bpr_sb = persist.tile([1, D], BF16)
        nc.sync.dma_start(bpr_sb[:], io["bpr"][:])
        b2r_sb = persist.tile([1, D], BF16)
        nc.sync.dma_start(b2r_sb[:], io["b2r"][:])
        colz_sb = persist.tile([128, 16], F32)
        nc.sync.dma_start(colz_sb[:], io["colz"][:])
        tri_sb = persist.tile([128, 2, 128], F32)
        for j in range(2):
            nc.sync.dma_start(tri_sb[:, j, :], io["tri"][:])
        if apply_bv:
            bvh2_sb = persist.tile([HD, 2], F32)
            nc.sync.dma_start(bvh2_sb[:], io["bvh2"][:])

        def ln_gb_tiles(gb_inp, nm):
            g_sb = persist.tile([128, D], F32, name=f"g_{nm}")
            b_sb = persist.tile([128, D], F32, name=f"b_{nm}")
            g_row = persist.tile([1, D], F32, name=f"gr_{nm}")
            b_row = persist.tile([1, D], F32, name=f"br_{nm}")
            nc.sync.dma_start(g_row[:], gb_inp[0:1, :])
            nc.sync.dma_start(b_row[:], gb_inp[1:2, :])
            nc.gpsimd.partition_broadcast(g_sb[:], g_row[:])
            nc.gpsimd.partition_broadcast(b_sb[:], b_row[:])
            return g_sb, b_sb

        ln1_g = ln1_b = ln2_g = ln2_b = None
        if apply_ln1_gb:
            ln1_g, ln1_b = ln_gb_tiles(io["ln1gb"], "ln1")
        if apply_ln2_gb:
            ln2_g, ln2_b = ln_gb_tiles(io["ln2gb"], "ln2")

        def layernorm(dst, src, pool, g_sb, b_sb):
            stats = pool.tile([128, 2, 6], F32, tag="ln_stats")
            sg = src.rearrange("p (g d) -> p g d", g=2)
            for g in range(2):
                nc.vector.bn_stats(out=stats[:, g, :], in_=sg[:, g, :])
            mv = pool.tile([128, 2], F32, tag="ln_mv")
            nc.vector.bn_aggr(out=mv[:], in_=stats[:])
            rstd = pool.tile([128, 1], F32, tag="ln_rstd")
            nc.scalar.activation(out=rstd[:], in_=mv[:, 1:2], func=AF.Sqrt,
                                 bias=eps_sb[:], scale=1.0)
            nc.vector.reciprocal(out=rstd[:], in_=rstd[:])
            nc.vector.tensor_scalar(out=dst, in0=src, scalar1=mv[:, 0:1],
                                    scalar2=rstd[:], op0=ALU.subtract,
                                    op1=ALU.mult)
            if g_sb is not None:
                nc.vector.tensor_mul(dst, dst, g_sb[:])
                nc.vector.tensor_add(dst, dst, b_sb[:])

        def transpose_into(dstT, src_tile, rt, tp_pool):
            for c in range(DC):
                tp = tp_pool.tile([128, 128], F32, tag="tp")
                nc.tensor.transpose(tp[:], src_tile[:, 128 * c:128 * (c + 1)],
                                    ident_sb[:])
                nc.scalar.copy(dstT[:, c, 128 * rt:128 * (rt + 1)], tp[:])

        # DRAM buffers for the AllToAlls; k/q ship as fp8e4 (halves the big
        # A2A; score noise washes out in softmax)
        kq_loc = dram.tile([NCORE, 2, 128, R], FP8)
        kq_g = dram.tile([NCORE, 2, 128, R], FP8)
        v_loc = dram.tile([NCORE, 128, CW], FP8)
        v_g = dram.tile([NCORE, 128, CW], FP8)
        o_loc = dram.tile([NCORE, 128, R], BF16)
        o_g = dram.tile([NCORE, 128, R], BF16)

        # preload the three projection weight matrices so the matmuls never
        # wait on HBM; DMA issue order (hs first, wk, then wv/wq) matters
        es_w = ctx.enter_context(contextlib.ExitStack())
        wq_pool = es_w.enter_context(tc.tile_pool(name="wqkv", bufs=1,
                                                  side="left"))

        def load_w(w_inp, nm):
            tiles = []
            for c in range(DC):
                wt = wq_pool.tile([128, D], BF16, name=f"w_{nm}_{c}")
                nc.sync.dma_start(wt[:], w_inp[128 * c:128 * (c + 1), :])
                tiles.append(wt)
            return tiles

        es_x = ctx.enter_context(contextlib.ExitStack())      # xT lifetime
        xT_pool = es_x.enter_context(
            tc.tile_pool(name="xT_p", bufs=1, side="left"))
        xT = xT_pool.tile([128, DC, R], BF16)

        hs_pool = ctx.enter_context(contextlib.ExitStack())   # hs_sb: P0..P5
        hsp = hs_pool.enter_context(tc.tile_pool(name="hs_p", bufs=1,
                                                 side="right"))
        hs_sb = hsp.tile([128, 4, D], F32)

        # ================= P0: load + LN1 + transpose ========================
        for rt in range(4):
            nc.sync.dma_start(hs_sb[:, rt, :], hs[128 * rt:128 * (rt + 1), :])
        wk_t = load_w(io["wk"], "wk")
        with tc.tile_pool(name="p0", bufs=2, side="left") as p0, \
             tc.tile_pool(name="p0ps", bufs=4, space="PSUM") as p0ps:
            for rt in range(4):
                xln = p0.tile([128, D], F32, tag="xln")
                layernorm(xln[:], hs_sb[:, rt, :], p0, ln1_g, ln1_b)
                transpose_into(xT, xln, rt, p0ps)
        wqt = load_w(io["wq"], "wq")
        wv_t = load_w(io["wv"], "wv")

        def proj_to_chunks(wts, bias_sb, scale, nm, dst, dt=BF16):
            """chunk oc of the projection -> DRAM dst(oc) ([128, R])."""
            with tc.tile_pool(name=f"t_{nm}", bufs=3, side="left") as tpl, \
                 tc.tile_pool(name=f"ps_{nm}", bufs=2, space="PSUM") as pps:
                for oc in range(DC):
                    ps = pps.tile([128, R], F32, tag="ps", name=f"ps_{nm}_{oc}")
                    for c in range(DC):
                        nc.tensor.matmul(
                            ps[:], wts[c][:, 128 * oc:128 * (oc + 1)],
                            xT[:, c, :], start=(c == 0), stop=(c == DC - 1))
                    tmp = tpl.tile([128, R], dt, tag="tmp",
                                   name=f"t_{nm}_{oc}")
                    nc.scalar.activation(tmp[:], ps[:], func=AF.Identity,
                                         bias=bias_sb[:, oc:oc + 1],
                                         scale=scale)
                    nc.sync.dma_start(dst(oc), tmp[:])

        # ================= P1: k-proj, q-proj; A2A(kq) =======================
        proj_to_chunks(wk_t, bkl_sb, 1.0, "wk", lambda oc: kq_loc[oc, 0],
                       dt=FP8)
        proj_to_chunks(wqt, bq8_sb, 0.125, "wq", lambda oc: kq_loc[oc, 1],
                       dt=FP8)
        nc.gpsimd.collective_compute(
            "AllToAll", ALU.bypass, replica_groups=GROUPS,
            ins=[kq_loc.opt()], outs=[kq_g.opt()])

        # ================= P2: v rows + ones cols; A2A(v) — last so the
        # kq A2A overlaps this projection =====================================
        with tc.tile_pool(name="vaug_p", bufs=1, side="right") as vaug_pool, \
             tc.tile_pool(name="ps_wv", bufs=2, space="PSUM") as pps:
            vaug = vaug_pool.tile([128, 4, VW], FP8)
            for pt in range(4):
                for cg in range(2):
                    ps = pps.tile([128, 512], F32, tag="ps",
                                  name=f"ps_wv_{pt}_{cg}")
                    for c in range(DC):
                        nc.tensor.matmul(
                            ps[:], xT[:, c, 128 * pt:128 * (pt + 1)],
                            wv_t[c][:, 512 * cg:512 * (cg + 1)],
                            start=(c == 0), stop=(c == DC - 1))
                    dst = vaug[:, pt, 520 * cg:520 * (cg + 1)].rearrange(
                        "p (h e) -> p h e", e=HD + 1)[:, :, 0:HD]
                    nc.scalar.copy(
                        dst, ps[:].rearrange("p (h e) -> p h e", e=HD))
                nc.sync.dma_start(
                    vaug[:, pt, :].rearrange("p (h e) -> p h e", e=HD + 1)
                    [:, :, HD:HD + 1],
                    io["vones"][:])
            for oc in range(DC):
                nc.sync.dma_start(
                    v_loc[oc].rearrange("p (t e) -> p t e", e=2 * (HD + 1)),
                    vaug[:, :, 2 * (HD + 1) * oc:2 * (HD + 1) * (oc + 1)])
        nc.gpsimd.collective_compute(
            "AllToAll", ALU.bypass, replica_groups=GROUPS,
            ins=[v_loc.opt()], outs=[v_g.opt()])
        es_x.close()   # xT no longer needed
        es_w.close()   # wk/wv/wq no longer needed

        # ================= P4: head-sharded causal attention =================
        es_wp = ctx.enter_context(contextlib.ExitStack())   # wp preload
        wp_pool = es_wp.enter_context(tc.tile_pool(name="w_wp", bufs=1,
                                                   side="left"))
        wp_t = []
        for c in range(DC):
            wt = wp_pool.tile([128, D], BF16, name=f"w_wp_{c}")
            nc.sync.dma_start(wt[:], io["wp"][128 * c:128 * (c + 1), :])
            wp_t.append(wt)

        es_attn = ctx.enter_context(contextlib.ExitStack())
        ao_pool = es_attn.enter_context(tc.tile_pool(name="ao_p", bufs=1,
                                                     side="left"))
        attn_oT = ao_pool.tile([128, DC, R], BF16)
        with tc.tile_pool(name="kg_p", bufs=1, side="left") as kgp, \
             tc.tile_pool(name="vg_p", bufs=1, side="left") as vgp, \
             tc.tile_pool(name="qg_p", bufs=1, side="left") as qgp, \
             tc.tile_pool(name="ex_p", bufs=4, side="left") as exp_pool, \
             tc.tile_pool(name="nrm_p", bufs=4, side="left") as nrm, \
             tc.tile_pool(name="sc_ps", bufs=2, space="PSUM") as scps, \
             tc.tile_pool(name="oT_ps", bufs=4, space="PSUM") as otps:
            kg = kgp.tile([128, NCORE, R], FP8)
            vg = vgp.tile([128, NCORE, 4, 2 * (HD + 1)], FP8)
            qg = qgp.tile([128, NCORE, R], FP8)
            for r in range(NCORE):
                nc.sync.dma_start(kg[:, r, :], kq_g[r, 0])
                nc.sync.dma_start(qg[:, r, :], kq_g[r, 1])
            for r in range(NCORE):
                nc.sync.dma_start(
                    vg[:, r, :, :],
                    v_g[r].rearrange("p (t e) -> p t e", e=2 * (HD + 1)))

            hps = (slice(0, 64), slice(64, 128))
            pending = []   # deferred av emissions; av lags sc by AV_LAG tiles
            AV_LAG = 2

            def emit_av(f):
                is_last, norm_f = f()
                if is_last:
                    norm_f()

            def flush_av(upto):
                while len(pending) > upto:
                    emit_av(pending.pop(0))

            for B2 in range(2):
                for a in (3, 2, 1, 0):
                    rq = 4 * B2 + a
                    oTs = [otps.tile([HD + 1, R], F32, tag="oT",
                                     name=f"oT_{B2}_{a}_{j}")
                           for j in range(2)]
                    nkt = 4 * a + 4
                    # diagonal tiles first (shrinking windows), then full
                    order = list(range(4 * a, nkt)) + list(range(4 * a))
                    exs = {}

                    def norm(oTs=oTs, B2=B2, a=a, rq=rq):
                        """normalize by the ones-column denominator, ship."""
                        o_sb = nrm.tile([128, R], BF16, tag="osb",
                                        name=f"osb_{B2}_{a}")
                        for j in range(2):
                            rec = nrm.tile([1, R], F32, tag="rec",
                                           name=f"rec_{B2}_{a}_{j}")
                            nc.vector.reciprocal(rec[:], oTs[j][HD:HD + 1, :])
                            rb = nrm.tile([HD, R], F32, tag="rb",
                                          name=f"rb_{B2}_{a}_{j}")
                            nc.gpsimd.partition_broadcast(rb[:], rec[:])
                            nc.vector.tensor_mul(o_sb[hps[j], :],
                                                 oTs[j][0:HD, :], rb[:])
                            if apply_bv:
                                nc.vector.tensor_scalar_add(
                                    o_sb[hps[j], :], o_sb[hps[j], :],
                                    bvh2_sb[:, j:j + 1])
                        nc.sync.dma_start(o_loc[rq], o_sb[:])

                    def av(i2, oTs=oTs, B2=B2, a=a, order=order, nkt=nkt,
                           exs=exs, norm=norm):
                        kt2 = order[i2]
                        rk2, t2 = 4 * B2 + kt2 // 4, kt2 % 4
                        d2 = kt2 - 4 * a
                        c2 = 128 * d2 if d2 >= 0 else 0
                        ex2 = exs.pop(i2)
                        for j in range(2):
                            nc.tensor.matmul(
                                oTs[j][:, c2:R],
                                vg[:, rk2, t2,
                                   (HD + 1) * j:(HD + 1) * (j + 1)],
                                ex2[:, j, c2:R],
                                start=(i2 == 0), stop=(i2 == nkt - 1))
                        return i2 == nkt - 1, norm

                    for i, kt in enumerate(order):
                        rk, t = 4 * B2 + kt // 4, kt % 4
                        d = kt - 4 * a
                        col0 = 128 * d if d >= 0 else 0
                        sc = scps.tile([128, 2, R], F32, tag="sc",
                                       name=f"sc_{B2}_{a}_{kt}")
                        for j in range(2):
                            nc.tensor.matmul(
                                sc[:, j, col0:R],
                                kg[hps[j], rk, 128 * t:128 * (t + 1)],
                                qg[hps[j], rq, col0:R],
                                start=True, stop=True)
                        if d >= 0:
                            nc.vector.tensor_add(sc[:, :, col0:col0 + 128],
                                                 sc[:, :, col0:col0 + 128],
                                                 tri_sb[:])
                        ex = exp_pool.tile([128, 2, R], FP8, tag="ex",
                                           name=f"ex_{B2}_{a}_{kt}")
                        nc.scalar.activation(ex[:, :, col0:R],
                                             sc[:, :, col0:R], func=AF.Exp,
                                             bias=colz_sb[:, kt:kt + 1],
                                             scale=1.0)
                        exs[i] = ex
                        pending.append(lambda i=i, av=av: av(i))
                        flush_av(AV_LAG)
            flush_av(0)

        nc.gpsimd.collective_compute(
            "AllToAll", ALU.bypass, replica_groups=GROUPS,
            ins=[o_loc.opt()], outs=[o_g.opt()])
        for c in range(DC):
            nc.sync.dma_start(attn_oT[:, c, :], o_g[c])

        # ================= P5: out-proj + residual ===========================
        es_h = ctx.enter_context(contextlib.ExitStack())      # h_sb: P5..P8
        h_pool = es_h.enter_context(tc.tile_pool(name="h_p", bufs=1,
                                                 side="right"))
        h_sb = h_pool.tile([128, 4, D], F32)
        es_mlp = ctx.enter_context(contextlib.ExitStack())    # h2T, gT
        mlp_pool = es_mlp.enter_context(tc.tile_pool(name="mlp_p", bufs=1,
                                                     side="right"))
        h2T = mlp_pool.tile([128, DC, R], BF16)
        gT = mlp_pool.tile([128, GC, R], BF16)
        with tc.tile_pool(name="ps_wp", bufs=2, space="PSUM") as pps, \
             tc.tile_pool(name="p6", bufs=2, side="left") as p6, \
             tc.tile_pool(name="p6ps", bufs=4, space="PSUM") as p6ps:
            for rt in range(4):
                for cg in range(2):
                    ps = pps.tile([128, 512], F32, tag="ps",
                                  name=f"ps_wp_{rt}_{cg}")
                    for c in range(DC):
                        nc.tensor.matmul(
                            ps[:], attn_oT[:, c, 128 * rt:128 * (rt + 1)],
                            wp_t[c][:, 512 * cg:512 * (cg + 1)],
                            start=(c == 0), stop=False)
                    nc.tensor.matmul(ps[:], ones_r[:],
                                     bpr_sb[:, 512 * cg:512 * (cg + 1)],
                                     start=False, stop=True)
                    nc.vector.tensor_add(h_sb[:, rt, 512 * cg:512 * (cg + 1)],
                                         ps[:],
                                         hs_sb[:, rt, 512 * cg:512 * (cg + 1)])
                # LN2 + transpose of this row-tile overlaps the next one's
                # projection matmuls
                h2 = p6.tile([128, D], F32, tag="h2")
                layernorm(h2[:], h_sb[:, rt, :], p6, ln2_g, ln2_b)
                transpose_into(h2T, h2, rt, p6ps)
        es_attn.close()  # attn_oT done
        es_wp.close()    # wp done

        # ================= P7: MLP up + gelu =================================
        with tc.tile_pool(name="w_w1", bufs=3, side="left") as wpl, \
             tc.tile_pool(name="ps_w1", bufs=2, space="PSUM") as pps:
            for gc in range(GC):
                wt = wpl.tile([128, DC, 128], BF16, tag="w1")
                nc.sync.dma_start(wt[:], io["w1"][gc])
                ps = pps.tile([128, R], F32, tag="ps", name=f"ps_w1_{gc}")
                for c in range(DC):
                    nc.tensor.matmul(ps[:], wt[:, c, :], h2T[:, c, :],
                                     start=(c == 0), stop=(c == DC - 1))
                nc.scalar.activation(gT[:, gc, :], ps[:], func=AF.Gelu,
                                     bias=b1l_sb[:, gc:gc + 1], scale=1.0)

        # ================= P8: MLP down + bias + residual ====================
        with tc.tile_pool(name="w_w2", bufs=3, side="left") as wpl, \
             tc.tile_pool(name="o_sb", bufs=2, side="left") as osb, \
             tc.tile_pool(name="o_ps", bufs=1, space="PSUM") as pps:
            psts = [pps.tile([128, 512], F32, tag=f"o{i}", name=f"o_ps_{i}")
                    for i in range(8)]
            for gc in range(GC):
                wt = wpl.tile([128, D], BF16, tag="w2")
                nc.sync.dma_start(wt[:], io["w2"][128 * gc:128 * (gc + 1), :])
                for qt in range(4):
                    for cg in range(2):
                        nc.tensor.matmul(
                            psts[2 * qt + cg][:],
                            gT[:, gc, 128 * qt:128 * (qt + 1)],
                            wt[:, 512 * cg:512 * (cg + 1)],
                            start=(gc == 0), stop=False)
            for qt in range(4):
                ot = osb.tile([128, D], F32, tag="ot", name=f"ot_{qt}")
                for cg in range(2):
                    nc.tensor.matmul(psts[2 * qt + cg][:], ones_r[:],
                                     b2r_sb[:, 512 * cg:512 * (cg + 1)],
                                     start=False, stop=True)
                    nc.vector.tensor_add(ot[:, 512 * cg:512 * (cg + 1)],
                                         psts[2 * qt + cg][:],
                                         h_sb[:, qt, 512 * cg:512 * (cg + 1)])
                nc.sync.dma_start(out[128 * qt:128 * (qt + 1), :], ot[:])


# ---------------------------------------------------------------------------
# Host side
# ---------------------------------------------------------------------------

_CACHE = {}
LAST_RESULT = None  # BassKernelResults of the most recent run (for test.py)


def _get_program(key):
    if key not in _CACHE:
        _CACHE[key] = build_program(*key)
    return _CACHE[key]


def kernel(hidden_states, Wq, bq, Wk, bk, Wv, bv, Wp, bp,
           ln1_g, ln1_b, ln2_g, ln2_b, W1, b1, W2, b2):
    f32 = lambda a: np.ascontiguousarray(np.asarray(a, dtype=np.float32))
    hidden_states = f32(hidden_states)
    Wq, bq, Wk, bk, Wv, bv, Wp, bp = map(f32, (Wq, bq, Wk, bk, Wv, bv, Wp, bp))
    ln1_g, ln1_b, ln2_g, ln2_b = map(f32, (ln1_g, ln1_b, ln2_g, ln2_b))
    W1, b1, W2, b2 = map(f32, (W1, b1, W2, b2))

    apply_bv = bool(np.any(bv != 0.0))
    apply_ln1 = bool(np.any(ln1_g != 1.0) or np.any(ln1_b != 0.0))
    apply_ln2 = bool(np.any(ln2_g != 1.0) or np.any(ln2_b != 0.0))
    nc = _get_program((apply_bv, apply_ln1, apply_ln2))

    chunk_major = lambda v: np.ascontiguousarray(v.reshape(-1, 128).T)

    # triangular mask: within a diagonal window, q-col j attends kpos p iff
    # j >= p
    p = np.arange(128)[:, None]
    j = np.arange(128)[None, :]
    tri = np.where(j >= p, np.float32(0.0), np.float32(NEG))

    # per-ktile column-zero exp bias: kpos = 128*kt + p
    kt = np.arange(16)[None, :]
    kpos = 128 * kt + p
    colz = np.where((kpos % JD) == (JD - 1), np.float32(NEG), np.float32(0.0))

    import ml_dtypes
    bf = lambda a: np.ascontiguousarray(a.astype(ml_dtypes.bfloat16))
    w1x = np.ascontiguousarray(
        W1.reshape(DC, 128, GC, 128).transpose(2, 1, 0, 3))
    shared = dict(wq=bf(Wq), wk=bf(Wk), wv=bf(Wv), wp=bf(Wp), w1=bf(w1x),
                  w2=bf(W2),
                  bq8=chunk_major(bq * 0.125), bkl=chunk_major(bk),
                  b1l=chunk_major(b1), bpr=bf(bp.reshape(1, D)),
                  b2r=bf(b2.reshape(1, D)), ln1gb=np.stack([ln1_g, ln1_b]),
                  ln2gb=np.stack([ln2_g, ln2_b]),
                  colz=np.ascontiguousarray(colz), tri=tri,
                  ident=np.eye(128, dtype=np.float32),
                  onesr=np.ones((1, 128), dtype=ml_dtypes.bfloat16),
                  vones=np.ones((128, H, 1), dtype=ml_dtypes.float8_e4m3fn))

    hs_flat = hidden_states.reshape(B * S, D)
    bvh = bv.reshape(H, HD).T  # [HD, H]
    in_maps = []
    for core in range(NCORE):
        m = dict(shared)
        m["hs"] = np.ascontiguousarray(hs_flat[R * core:R * (core + 1)])
        m["bvh2"] = np.ascontiguousarray(bvh[:, 2 * core:2 * core + 2])
        in_maps.append(m)

    res = run_bass_kernel_spmd(nc, in_maps, core_ids=list(range(NCORE)))
    global LAST_RESULT
    LAST_RESULT = res

    out_full = np.empty((B * S, D), dtype=np.float32)
    for core in range(NCORE):
        out_full[R * core:R * (core + 1)] = res.results[core]["out"]
    return out_full.reshape(B, S, D)


# revision 54
# speedup vs baseline: 1.0105x; 1.0105x over previous
"""Trainium2 Bass kernel for nn_Block_55336358643145 (dense transformer block).

v2: head-sharded exact-causal attention.

Row-shards the 4096 (batch*seq) rows contiguously: core c owns rows
512c..512(c+1) (cores 0-3 batch 0, 4-7 batch 1). Per core: LN1 ->
transpose -> Q/K/V projections emitted in output-chunk-major order so
chunk oc (128 dims = heads 2oc,2oc+1) is the AllToAll payload for core
oc. Two AllToAlls (k+v early, q after) re-shard to head-parallel: core j
holds q/k/v for heads 2j,2j+1 over all 4096 rows. Attention is then an
exact causal triangle, identical on every core (shared triangular mask,
shared per-ktile column-zero exp bias, shrinking q-windows on diagonal
tiles), with a per-head ones column in v producing the softmax
denominator. Normalized outputs AllToAll back to row-sharding, then
out-proj + residual, LN2, 4x MLP with exact-erf Gelu as before. All
matmuls bf16.
"""

import contextlib

import numpy as np

import concourse.bass as bass
import concourse.tile as tile
from concourse import bacc, mybir
from concourse.bass_utils import run_bass_kernel_spmd

F32 = mybir.dt.float32
BF16 = mybir.dt.bfloat16
FP8 = mybir.dt.float8e4
AF = mybir.ActivationFunctionType
ALU = mybir.AluOpType

B, S, D, H, HD, FF = 2, 2048, 1024, 16, 64, 4096
NCORE = 8
R = 512            # rows per core
DC = D // 128      # 8 d-chunks
GC = FF // 128     # 32 mlp hidden chunks
VW = H * (HD + 1)  # 1040: v with per-head ones column
CW = 4 * (HD + 1) * 2  # 520: v cols per partition in an A2A chunk
LN_EPS = 1e-5
JD = 25            # joined dim for the column-zero mask
NEG = -1.0e30
GROUPS = [[0, 1, 2, 3, 4, 5, 6, 7]]


def build_program(apply_bv, apply_ln1_gb, apply_ln2_gb):
    nc = bacc.Bacc("TRN2", target_bir_lowering=False, debug=False,
                   num_devices=NCORE)

    def inp(name, shape):
        return nc.dram_tensor(name, list(shape), F32, kind="ExternalInput").ap()

    def binp(name, shape):
        return nc.dram_tensor(name, list(shape), BF16,
                              kind="ExternalInput").ap()

    io = dict(
        hs=inp("hs", (R, D)),
        wq=binp("wq", (D, D)), wk=binp("wk", (D, D)),
        wv=binp("wv", (D, D)), wp=binp("wp", (D, D)),
        w1=binp("w1", (GC, 128, DC, 128)), w2=binp("w2", (FF, D)),
        bq8=inp("bq8", (128, DC)), bkl=inp("bkl", (128, DC)),
        bvh2=inp("bvh2", (HD, 2)), b1l=inp("b1l", (128, GC)),
        bpr=binp("bpr", (1, D)), b2r=binp("b2r", (1, D)),
        ln1gb=inp("ln1gb", (2, D)), ln2gb=inp("ln2gb", (2, D)),
        colz=inp("colz", (128, 16)),
        tri=inp("tri", (128, 128)),
        ident=inp("ident", (128, 128)),
        onesr=binp("onesr", (1, 128)),
        vones=nc.dram_tensor("vones", [128, H, 1], BF16,
                             kind="ExternalInput").ap(),
        out=nc.dram_tensor("out", [R, D], F32, kind="ExternalOutput").ap(),
    )

    with tile.TileContext(nc) as tc:
        _build(tc, io, apply_bv, apply_ln1_gb, apply_ln2_gb)
    nc.compile()
    return nc


def _build(tc, io, apply_bv, apply_ln1_gb, apply_ln2_gb):
    nc = tc.nc
    hs, out = io["hs"], io["out"]

    with contextlib.ExitStack() as ctx:
        persist = ctx.enter_context(tc.tile_pool(name="persist", bufs=1,
                                                 side="left"))
        dram = ctx.enter_context(tc.tile_pool(name="dram", bufs=1,
                                              space="DRAM"))

        # ---- small constants ------------------------------------------------
        ident_sb = persist.tile([128, 128], F32)
        nc.sync.dma_start(ident_sb[:], io["ident"][:])
        eps_sb = persist.tile([128, 1], F32)
        nc.vector.memset(eps_sb[:], LN_EPS)
        ones_r = persist.tile([1, 128], BF16)
        nc.sync.dma_start(ones_r[:], io["onesr"][:])
        bq8_sb = persist.tile([128, DC], F32)
        nc.sync.dma_start(bq8_sb[:], io["bq8"][:])
        bkl_sb = persist.tile([128, DC], F32)
        nc.sync.dma_start(bkl_sb[:], io["bkl"][:])
        b1l_sb = persist.tile([128, GC], F32)
        nc.sync.dma_start(b1l_sb[:], io["b1l"][:])
        bpr_sb = persist.tile([1, D], BF16)
        nc.sync.dma_start(bpr_sb[:], io["bpr"][:])
        b2r_sb = persist.tile([1, D], BF16)
        nc.sync.dma_start(b2r_sb[:], io["b2r"][:])
        colz_sb = persist.tile([128, 16], F32)
        nc.sync.dma_start(colz_sb[:], io["colz"][:])
        tri_sb = persist.tile([128, 2, 128], F32)
        for j in range(2):
            nc.sync.dma_start(tri_sb[:, j, :], io["tri"][:])
        if apply_bv:
            bvh2_sb = persist.tile([HD, 2], F32)
            nc.sync.dma_start(bvh2_sb[:], io["bvh2"][:])

        def ln_gb_tiles(gb_inp, nm):
            g_sb = persist.tile([128, D], F32, name=f"g_{nm}")
            b_sb = persist.tile([128, D], F32, name=f"b_{nm}")
            g_row = persist.tile([1, D], F32, name=f"gr_{nm}")
            b_row = persist.tile([1, D], F32, name=f"br_{nm}")
            nc.sync.dma_start(g_row[:], gb_inp[0:1, :])
            nc.sync.dma_start(b_row[:], gb_inp[1:2, :])
            nc.gpsimd.partition_broadcast(g_sb[:], g_row[:])
            nc.gpsimd.partition_broadcast(b_sb[:], b_row[:])
            return g_sb, b_sb

        ln1_g = ln1_b = ln2_g = ln2_b = None
        if apply_ln1_gb:
            ln1_g, ln1_b = ln_gb_tiles(io["ln1gb"], "ln1")
        if apply_ln2_gb:
            ln2_g, ln2_b = ln_gb_tiles(io["ln2gb"], "ln2")

        def layernorm(dst, src, pool, g_sb, b_sb):
            stats = pool.tile([128, 2, 6], F32, tag="ln_stats")
            sg = src.rearrange("p (g d) -> p g d", g=2)
            for g in range(2):
                nc.vector.bn_stats(out=stats[:, g, :], in_=sg[:, g, :])
            mv = pool.tile([128, 2], F32, tag="ln_mv")
            nc.vector.bn_aggr(out=mv[:], in_=stats[:])
            rstd = pool.tile([128, 1], F32, tag="ln_rstd")
            nc.scalar.activation(out=rstd[:], in_=mv[:, 1:2], func=AF.Sqrt,
                                 bias=eps_sb[:], scale=1.0)
            nc.vector.reciprocal(out=rstd[:], in_=rstd[:])
            nc.vector.tensor_scalar(out=dst, in0=src, scalar1=mv[:, 0:1],
                                    scalar2=rstd[:], op0=ALU.subtract,
                                    op1=ALU.mult)
            if g_sb is not None:
                nc.vector.tensor_mul(dst, dst, g_sb[:])
                nc.vector.tensor_add(dst, dst, b_sb[:])

        def transpose_into(dstT, src_tile, rt, tp_pool):
            for c in range(DC):
                tp = tp_pool.tile([128, 128], F32, tag="tp")
                nc.tensor.transpose(tp[:], src_tile[:, 128 * c:128 * (c + 1)],
                                    ident_sb[:])
                nc.scalar.copy(dstT[:, c, 128 * rt:128 * (rt + 1)], tp[:])

        # DRAM buffers for the AllToAlls; k/q ship as fp8e4 (halves the big
        # A2A; score noise washes out in softmax)
        kq_loc = dram.tile([NCORE, 2, 128, R], FP8)
        kq_g = dram.tile([NCORE, 2, 128, R], FP8)
        v_loc = dram.tile([NCORE, 128, CW], BF16)
        v_g = dram.tile([NCORE, 128, CW], BF16)
        o_loc = dram.tile([NCORE, 128, R], BF16)
        o_g = dram.tile([NCORE, 128, R], BF16)

        # preload the three projection weight matrices so the matmuls never
        # wait on HBM; DMA issue order (hs first, wk, then wv/wq) matters
        es_w = ctx.enter_context(contextlib.ExitStack())
        wq_pool = es_w.enter_context(tc.tile_pool(name="wqkv", bufs=1,
                                                  side="left"))

        def load_w(w_inp, nm):
            tiles = []
            for c in range(DC):
                wt = wq_pool.tile([128, D], BF16, name=f"w_{nm}_{c}")
                nc.sync.dma_start(wt[:], w_inp[128 * c:128 * (c + 1), :])
                tiles.append(wt)
            return tiles

        es_x = ctx.enter_context(contextlib.ExitStack())      # xT lifetime
        xT_pool = es_x.enter_context(
            tc.tile_pool(name="xT_p", bufs=1, side="left"))
        xT = xT_pool.tile([128, DC, R], BF16)

        hs_pool = ctx.enter_context(contextlib.ExitStack())   # hs_sb: P0..P5
        hsp = hs_pool.enter_context(tc.tile_pool(name="hs_p", bufs=1,
                                                 side="right"))
        hs_sb = hsp.tile([128, 4, D], F32)

        # ================= P0: load + LN1 + transpose ========================
        for rt in range(4):
            nc.sync.dma_start(hs_sb[:, rt, :], hs[128 * rt:128 * (rt + 1), :])
        wk_t = load_w(io["wk"], "wk")
        with tc.tile_pool(name="p0", bufs=2, side="left") as p0, \
             tc.tile_pool(name="p0ps", bufs=4, space="PSUM") as p0ps:
            for rt in range(4):
                xln = p0.tile([128, D], F32, tag="xln")
                layernorm(xln[:], hs_sb[:, rt, :], p0, ln1_g, ln1_b)
                transpose_into(xT, xln, rt, p0ps)
        wqt = load_w(io["wq"], "wq")
        wv_t = load_w(io["wv"], "wv")

        def proj_to_chunks(wts, bias_sb, scale, nm, dst, dt=BF16):
            """chunk oc of the projection -> DRAM dst(oc) ([128, R])."""
            with tc.tile_pool(name=f"t_{nm}", bufs=3, side="left") as tpl, \
                 tc.tile_pool(name=f"ps_{nm}", bufs=2, space="PSUM") as pps:
                for oc in range(DC):
                    ps = pps.tile([128, R], F32, tag="ps", name=f"ps_{nm}_{oc}")
                    for c in range(DC):
                        nc.tensor.matmul(
                            ps[:], wts[c][:, 128 * oc:128 * (oc + 1)],
                            xT[:, c, :], start=(c == 0), stop=(c == DC - 1))
                    tmp = tpl.tile([128, R], dt, tag="tmp",
                                   name=f"t_{nm}_{oc}")
                    nc.scalar.activation(tmp[:], ps[:], func=AF.Identity,
                                         bias=bias_sb[:, oc:oc + 1],
                                         scale=scale)
                    nc.sync.dma_start(dst(oc), tmp[:])

        # ================= P1: k-proj, q-proj; A2A(kq) =======================
        proj_to_chunks(wk_t, bkl_sb, 1.0, "wk", lambda oc: kq_loc[oc, 0],
                       dt=FP8)
        proj_to_chunks(wqt, bq8_sb, 0.125, "wq", lambda oc: kq_loc[oc, 1],
                       dt=FP8)
        nc.gpsimd.collective_compute(
            "AllToAll", ALU.bypass, replica_groups=GROUPS,
            ins=[kq_loc.opt()], outs=[kq_g.opt()])

        # ================= P2: v rows + ones cols; A2A(v) — last so the
        # kq A2A overlaps this projection =====================================
        with tc.tile_pool(name="vaug_p", bufs=1, side="right") as vaug_pool, \
             tc.tile_pool(name="ps_wv", bufs=2, space="PSUM") as pps:
            vaug = vaug_pool.tile([128, 4, VW], BF16)
            for pt in range(4):
                for cg in range(2):
                    ps = pps.tile([128, 512], F32, tag="ps",
                                  name=f"ps_wv_{pt}_{cg}")
                    for c in range(DC):
                        nc.tensor.matmul(
                            ps[:], xT[:, c, 128 * pt:128 * (pt + 1)],
                            wv_t[c][:, 512 * cg:512 * (cg + 1)],
                            start=(c == 0), stop=(c == DC - 1))
                    dst = vaug[:, pt, 520 * cg:520 * (cg + 1)].rearrange(
                        "p (h e) -> p h e", e=HD + 1)[:, :, 0:HD]
                    nc.scalar.copy(
                        dst, ps[:].rearrange("p (h e) -> p h e", e=HD))
                nc.sync.dma_start(
                    vaug[:, pt, :].rearrange("p (h e) -> p h e", e=HD + 1)
                    [:, :, HD:HD + 1],
                    io["vones"][:])
            for oc in range(DC):
                nc.sync.dma_start(
                    v_loc[oc].rearrange("p (t e) -> p t e", e=2 * (HD + 1)),
                    vaug[:, :, 2 * (HD + 1) * oc:2 * (HD + 1) * (oc + 1)])
        nc.gpsimd.collective_compute(
            "AllToAll", ALU.bypass, replica_groups=GROUPS,
            ins=[v_loc.opt()], outs=[v_g.opt()])
        es_x.close()   # xT no longer needed
        es_w.close()   # wk/wv/wq no longer needed

        # ================= P4: head-sharded causal attention =================
        es_wp = ctx.enter_context(contextlib.ExitStack())   # wp preload
        wp_pool = es_wp.enter_context(tc.tile_pool(name="w_wp", bufs=1,
                                                   side="left"))
        wp_t = []
        for c in range(DC):
            wt = wp_pool.tile([128, D], BF16, name=f"w_wp_{c}")
            nc.sync.dma_start(wt[:], io["wp"][128 * c:128 * (c + 1), :])
            wp_t.append(wt)

        es_attn = ctx.enter_context(contextlib.ExitStack())
        ao_pool = es_attn.enter_context(tc.tile_pool(name="ao_p", bufs=1,
                                                     side="left"))
        attn_oT = ao_pool.tile([128, DC, R], BF16)
        with tc.tile_pool(name="kg_p", bufs=1, side="left") as kgp, \
             tc.tile_pool(name="vg_p", bufs=1, side="left") as vgp, \
             tc.tile_pool(name="qg_p", bufs=1, side="left") as qgp, \
             tc.tile_pool(name="ex_p", bufs=4, side="left") as exp_pool, \
             tc.tile_pool(name="nrm_p", bufs=4, side="left") as nrm, \
             tc.tile_pool(name="sc_ps", bufs=2, space="PSUM") as scps, \
             tc.tile_pool(name="oT_ps", bufs=4, space="PSUM") as otps:
            kg = kgp.tile([128, NCORE, R], FP8)
            vg = vgp.tile([128, NCORE, 4, 2 * (HD + 1)], BF16)
            qg = qgp.tile([128, NCORE, R], FP8)
            for r in range(NCORE):
                nc.sync.dma_start(kg[:, r, :], kq_g[r, 0])
                nc.sync.dma_start(qg[:, r, :], kq_g[r, 1])
            for r in range(NCORE):
                nc.sync.dma_start(
                    vg[:, r, :, :],
                    v_g[r].rearrange("p (t e) -> p t e", e=2 * (HD + 1)))

            hps = (slice(0, 64), slice(64, 128))
            pending = []   # deferred av emissions; av lags sc by AV_LAG tiles
            AV_LAG = 2

            def emit_av(f):
                is_last, norm_f = f()
                if is_last:
                    norm_f()

            def flush_av(upto):
                while len(pending) > upto:
                    emit_av(pending.pop(0))

            for B2 in range(2):
                for a in (3, 2, 1, 0):
                    rq = 4 * B2 + a
                    oTs = [otps.tile([HD + 1, R], F32, tag="oT",
                                     name=f"oT_{B2}_{a}_{j}")
                           for j in range(2)]
                    nkt = 4 * a + 4
                    # diagonal tiles first (shrinking windows), then full
                    order = list(range(4 * a, nkt)) + list(range(4 * a))
                    exs = {}

                    def norm(oTs=oTs, B2=B2, a=a, rq=rq):
                        """normalize by the ones-column denominator, ship."""
                        o_sb = nrm.tile([128, R], BF16, tag="osb",
                                        name=f"osb_{B2}_{a}")
                        for j in range(2):
                            rec = nrm.tile([1, R], F32, tag="rec",
                                           name=f"rec_{B2}_{a}_{j}")
                            nc.vector.reciprocal(rec[:], oTs[j][HD:HD + 1, :])
                            rb = nrm.tile([HD, R], F32, tag="rb",
                                          name=f"rb_{B2}_{a}_{j}")
                            nc.gpsimd.partition_broadcast(rb[:], rec[:])
                            nc.vector.tensor_mul(o_sb[hps[j], :],
                                                 oTs[j][0:HD, :], rb[:])
                            if apply_bv:
                                nc.vector.tensor_scalar_add(
                                    o_sb[hps[j], :], o_sb[hps[j], :],
                                    bvh2_sb[:, j:j + 1])
                        nc.sync.dma_start(o_loc[rq], o_sb[:])

                    def av(i2, oTs=oTs, B2=B2, a=a, order=order, nkt=nkt,
                           exs=exs, norm=norm):
                        kt2 = order[i2]
                        rk2, t2 = 4 * B2 + kt2 // 4, kt2 % 4
                        d2 = kt2 - 4 * a
                        c2 = 128 * d2 if d2 >= 0 else 0
                        ex2 = exs.pop(i2)
                        for j in range(2):
                            nc.tensor.matmul(
                                oTs[j][:, c2:R],
                                vg[:, rk2, t2,
                                   (HD + 1) * j:(HD + 1) * (j + 1)],
                                ex2[:, j, c2:R],
                                start=(i2 == 0), stop=(i2 == nkt - 1))
                        return i2 == nkt - 1, norm

                    for i, kt in enumerate(order):
                        rk, t = 4 * B2 + kt // 4, kt % 4
                        d = kt - 4 * a
                        col0 = 128 * d if d >= 0 else 0
                        sc = scps.tile([128, 2, R], F32, tag="sc",
                                       name=f"sc_{B2}_{a}_{kt}")
                        for j in range(2):
                            nc.tensor.matmul(
                                sc[:, j, col0:R],
                                kg[hps[j], rk, 128 * t:128 * (t + 1)],
                                qg[hps[j], rq, col0:R],
                                start=True, stop=True)
                        if d >= 0:
                            nc.vector.tensor_add(sc[:, :, col0:col0 + 128],
                                                 sc[:, :, col0:col0 + 128],
                                                 tri_sb[:])
                        ex = exp_pool.tile([128, 2, R], BF16, tag="ex",
                                           name=f"ex_{B2}_{a}_{kt}")
                        nc.scalar.activation(ex[:, :, col0:R],
                                             sc[:, :, col0:R], func=AF.Exp,
                                             bias=colz_sb[:, kt:kt + 1],
                                             scale=1.0)
                        exs[i] = ex
                        pending.append(lambda i=i, av=av: av(i))
                        flush_av(AV_LAG)
            flush_av(0)

        nc.gpsimd.collective_compute(
            "AllToAll", ALU.bypass, replica_groups=GROUPS,
            ins=[o_loc.opt()], outs=[o_g.opt()])
        for c in range(DC):
            nc.sync.dma_start(attn_oT[:, c, :], o_g[c])

        # ================= P5: out-proj + residual ===========================
        es_h = ctx.enter_context(contextlib.ExitStack())      # h_sb: P5..P8
        h_pool = es_h.enter_context(tc.tile_pool(name="h_p", bufs=1,
                                                 side="right"))
        h_sb = h_pool.tile([128, 4, D], F32)
        es_mlp = ctx.enter_context(contextlib.ExitStack())    # h2T, gT
        mlp_pool = es_mlp.enter_context(tc.tile_pool(name="mlp_p", bufs=1,
                                                     side="right"))
        h2T = mlp_pool.tile([128, DC, R], BF16)
        gT = mlp_pool.tile([128, GC, R], BF16)
        with tc.tile_pool(name="ps_wp", bufs=2, space="PSUM") as pps, \
             tc.tile_pool(name="p6", bufs=2, side="left") as p6, \
             tc.tile_pool(name="p6ps", bufs=4, space="PSUM") as p6ps:
            for rt in range(4):
                for cg in range(2):
                    ps = pps.tile([128, 512], F32, tag="ps",
                                  name=f"ps_wp_{rt}_{cg}")
                    for c in range(DC):
                        nc.tensor.matmul(
                            ps[:], attn_oT[:, c, 128 * rt:128 * (rt + 1)],
                            wp_t[c][:, 512 * cg:512 * (cg + 1)],
                            start=(c == 0), stop=False)
                    nc.tensor.matmul(ps[:], ones_r[:],
                                     bpr_sb[:, 512 * cg:512 * (cg + 1)],
                                     start=False, stop=True)
                    nc.vector.tensor_add(h_sb[:, rt, 512 * cg:512 * (cg + 1)],
                                         ps[:],
                                         hs_sb[:, rt, 512 * cg:512 * (cg + 1)])
                # LN2 + transpose of this row-tile overlaps the next one's
                # projection matmuls
                h2 = p6.tile([128, D], F32, tag="h2")
                layernorm(h2[:], h_sb[:, rt, :], p6, ln2_g, ln2_b)
                transpose_into(h2T, h2, rt, p6ps)
        es_attn.close()  # attn_oT done
        es_wp.close()    # wp done

        # ================= P7: MLP up + gelu =================================
        with tc.tile_pool(name="w_w1", bufs=3, side="left") as wpl, \
             tc.tile_pool(name="ps_w1", bufs=2, space="PSUM") as pps:
            for gc in range(GC):
                wt = wpl.tile([128, DC, 128], BF16, tag="w1")
                nc.sync.dma_start(wt[:], io["w1"][gc])
                ps = pps.tile([128, R], F32, tag="ps", name=f"ps_w1_{gc}")
                for c in range(DC):
                    nc.tensor.matmul(ps[:], wt[:, c, :], h2T[:, c, :],
                                     start=(c == 0), stop=(c == DC - 1))
                nc.scalar.activation(gT[:, gc, :], ps[:], func=AF.Gelu,
                                     bias=b1l_sb[:, gc:gc + 1], scale=1.0)

        # ================= P8: MLP down + bias + residual ====================
        with tc.tile_pool(name="w_w2", bufs=3, side="left") as wpl, \
             tc.tile_pool(name="o_sb", bufs=2, side="left") as osb, \
             tc.tile_pool(name="o_ps", bufs=1, space="PSUM") as pps:
            psts = [pps.tile([128, 512], F32, tag=f"o{i}", name=f"o_ps_{i}")
                    for i in range(8)]
            for gc in range(GC):
                wt = wpl.tile([128, D], BF16, tag="w2")
                nc.sync.dma_start(wt[:], io["w2"][128 * gc:128 * (gc + 1), :])
                for qt in range(4):
                    for cg in range(2):
                        nc.tensor.matmul(
                            psts[2 * qt + cg][:],
                            gT[:, gc, 128 * qt:128 * (qt + 1)],
                            wt[:, 512 * cg:512 * (cg + 1)],
                            start=(gc == 0), stop=False)
            for qt in range(4):
                ot = osb.tile([128, D], F32, tag="ot", name=f"ot_{qt}")
                for cg in range(2):
                    nc.tensor.matmul(psts[2 * qt + cg][:], ones_r[:],
                                     b2r_sb[:, 512 * cg:512 * (cg + 1)],
                                     start=False, stop=True)
                    nc.vector.tensor_add(ot[:, 512 * cg:512 * (cg + 1)],
                                         psts[2 * qt + cg][:],
                                         h_sb[:, qt, 512 * cg:512 * (cg + 1)])
                nc.sync.dma_start(out[128 * qt:128 * (qt + 1), :], ot[:])


# ---------------------------------------------------------------------------
# Host side
# ---------------------------------------------------------------------------

_CACHE = {}
LAST_RESULT = None  # BassKernelResults of the most recent run (for test.py)


def _get_program(key):
    if key not in _CACHE:
        _CACHE[key] = build_program(*key)
    return _CACHE[key]


def kernel(hidden_states, Wq, bq, Wk, bk, Wv, bv, Wp, bp,
           ln1_g, ln1_b, ln2_g, ln2_b, W1, b1, W2, b2):
    f32 = lambda a: np.ascontiguousarray(np.asarray(a, dtype=np.float32))
    hidden_states = f32(hidden_states)
    Wq, bq, Wk, bk, Wv, bv, Wp, bp = map(f32, (Wq, bq, Wk, bk, Wv, bv, Wp, bp))
    ln1_g, ln1_b, ln2_g, ln2_b = map(f32, (ln1_g, ln1_b, ln2_g, ln2_b))
    W1, b1, W2, b2 = map(f32, (W1, b1, W2, b2))

    apply_bv = bool(np.any(bv != 0.0))
    apply_ln1 = bool(np.any(ln1_g != 1.0) or np.any(ln1_b != 0.0))
    apply_ln2 = bool(np.any(ln2_g != 1.0) or np.any(ln2_b != 0.0))
    nc = _get_program((apply_bv, apply_ln1, apply_ln2))

    chunk_major = lambda v: np.ascontiguousarray(v.reshape(-1, 128).T)

    # triangular mask: within a diagonal window, q-col j attends kpos p iff
    # j >= p
    p = np.arange(128)[:, None]
    j = np.arange(128)[None, :]
    tri = np.where(j >= p, np.float32(0.0), np.float32(NEG))

    # per-ktile column-zero exp bias: kpos = 128*kt + p
    kt = np.arange(16)[None, :]
    kpos = 128 * kt + p
    colz = np.where((kpos % JD) == (JD - 1), np.float32(NEG), np.float32(0.0))

    import ml_dtypes
    bf = lambda a: np.ascontiguousarray(a.astype(ml_dtypes.bfloat16))
    w1x = np.ascontiguousarray(
        W1.reshape(DC, 128, GC, 128).transpose(2, 1, 0, 3))
    shared = dict(wq=bf(Wq), wk=bf(Wk), wv=bf(Wv), wp=bf(Wp), w1=bf(w1x),
                  w2=bf(W2),
                  bq8=chunk_major(bq * 0.125), bkl=chunk_major(bk),
                  b1l=chunk_major(b1), bpr=bf(bp.reshape(1, D)),
                  b2r=bf(b2.reshape(1, D)), ln1gb=np.stack([ln1_g, ln1_b]),
                  ln2gb=np.stack([ln2_g, ln2_b]),
                  colz=np.ascontiguousarray(colz), tri=tri,
                  ident=np.eye(128, dtype=np.float32),
                  onesr=np.ones((1, 128), dtype=ml_dtypes.bfloat16),
                  vones=np.ones((128, H, 1), dtype=ml_dtypes.bfloat16))

    hs_flat = hidden_states.reshape(B * S, D)
    bvh = bv.reshape(H, HD).T  # [HD, H]
    in_maps = []
    for core in range(NCORE):
        m = dict(shared)
        m["hs"] = np.ascontiguousarray(hs_flat[R * core:R * (core + 1)])
        m["bvh2"] = np.ascontiguousarray(bvh[:, 2 * core:2 * core + 2])
        in_maps.append(m)

    res = run_bass_kernel_spmd(nc, in_maps, core_ids=list(range(NCORE)))
    global LAST_RESULT
    LAST_RESULT = res

    out_full = np.empty((B * S, D), dtype=np.float32)
    for core in range(NCORE):
        out_full[R * core:R * (core + 1)] = res.results[core]["out"]
    return out_full.reshape(B, S, D)
